# revision 10
# baseline (speedup 1.0000x reference)
"""Multi-head attention (B=4, S=2048, C=1024, H=16) on 8 TRN2 NeuronCores.

Sharding: data-parallel over batch (4) x query-row split (2). Core c handles
batch c//2, query rows [(c%2)*1024, (c%2)*1024+1024). Each core computes the
QKV projection for its batch (K/V over the full sequence, Q over its own rows)
with float32r matmuls, spills Q^T/K^T/V to DRAM scratch, then runs per-head
attention with transposed scores ([key, query] layout) so the softmaxed
probabilities feed the P.V matmul directly as the moving operand. A ones
column appended to V accumulates the softmax denominator in the same PSUM
tile. The out-projection uses O^T as the stationary operand so the result is
produced in natural [row, channel] layout. No collectives.
"""

from contextlib import ExitStack

import numpy as np

import concourse.bass as bass
import concourse.mybir as mybir
import concourse.tile as tile
from concourse import bacc
from concourse.bass_utils import run_bass_kernel_spmd
from concourse.masks import make_identity

F32 = mybir.dt.float32
F32R = mybir.dt.float32r
AF = mybir.ActivationFunctionType

B, S, C, H, DH = 4, 2048, 1024, 16, 64
NCORES = 8
SCALE = DH ** -0.5  # 0.125
CT = C // 128  # 8 channel tiles
ST = S // 128  # 16 seq tiles
MYROWS = S // 2  # 1024 query rows per core


def _transpose_group(nc, tp_pool, ident, src, cts, dst_ap):
    """PE-transpose 4 [128,128] blocks of src (channel tiles cts) and copy the
    [128, 512] group to dst_ap (a strided AP covering the 4 destinations)."""
    tp = tp_pool.tile([128, 512], F32)
    for k, ct in enumerate(cts):
        nc.tensor.transpose(tp[:, k * 128:(k + 1) * 128],
                            src[:, ct * 128:(ct + 1) * 128], ident)
    nc.vector.tensor_copy(dst_ap, tp[:])


def build():
    nc = bacc.Bacc("TRN2", target_bir_lowering=False, debug=False,
                   num_devices=NCORES)

    x = nc.dram_tensor("x", [S, C], F32, kind="ExternalInput").ap()
    w_qkv = nc.dram_tensor("w_qkv", [3 * C, C], F32, kind="ExternalInput").ap()
    b_qkv = nc.dram_tensor("b_qkv", [3 * C], F32, kind="ExternalInput").ap()
    w_out = nc.dram_tensor("w_out", [C, C], F32, kind="ExternalInput").ap()
    b_out = nc.dram_tensor("b_out", [C], F32, kind="ExternalInput").ap()
    out = nc.dram_tensor("out", [MYROWS, C], F32, kind="ExternalOutput").ap()

    # DRAM scratch for the projected tensors (feature-major Q^T/K^T, natural V)
    qT_d = nc.dram_tensor("qT_d", [C, MYROWS], F32R).ap()
    kT_d = nc.dram_tensor("kT_d", [C, S], F32R).ap()
    v_d = nc.dram_tensor("v_d", [S, C], F32R).ap()

    with tile.TileContext(nc) as tc, ExitStack() as ctx:
        const = ctx.enter_context(tc.tile_pool(name="const", bufs=1))
        ident = const.tile([128, 128], F32)
        make_identity(nc, ident[:])

        b_sb = const.tile([128, 3 * C // 128], F32)  # b_sb[p, wt] = b_qkv[wt*128+p]
        nc.sync.dma_start(b_sb[:], b_qkv.rearrange("(wt p) -> p wt", p=128))
        bo_sb = const.tile([1, C], F32)
        nc.sync.dma_start(bo_sb[:], b_out[None, :])
        bo_bc = const.tile([128, C], F32)
        nc.gpsimd.partition_broadcast(bo_bc[:], bo_sb[0:1, :])

        persist = ctx.enter_context(tc.tile_pool(name="persist", bufs=1))
        xT = persist.tile([128, CT * S], F32R)  # xT[p, ct*S + s] = x[s, ct*128+p]

        # ---------------- Phase A: transposes + QKV projection ----------------
        with ExitStack() as actx:
            xnat = actx.enter_context(tc.tile_pool(name="xnat", bufs=3))
            wnat = actx.enter_context(tc.tile_pool(name="wnat", bufs=3))
            wstrip = actx.enter_context(tc.tile_pool(name="wstrip", bufs=3))
            vw_pool = actx.enter_context(tc.tile_pool(name="vw", bufs=1))
            stage = actx.enter_context(tc.tile_pool(name="stage", bufs=4))
            tp_ps = actx.enter_context(
                tc.tile_pool(name="tp_ps", bufs=3, space="PSUM"))
            acc_ps = actx.enter_context(
                tc.tile_pool(name="acc_ps", bufs=2, space="PSUM"))

            # x^T
            for st in range(ST):
                xn = xnat.tile([128, C], F32)
                nc.sync.dma_start(xn[:], x[st * 128:(st + 1) * 128, :])
                for g in range(2):
                    cts = range(g * 4, g * 4 + 4)
                    # dst: xT[:, ct*S + st*128 : +128] for ct in cts
                    dst = xT[:].rearrange("p (ct s) -> p ct s", ct=CT)[
                        :, g * 4:g * 4 + 4, st * 128:(st + 1) * 128]
                    _transpose_group(nc, tp_ps, ident[:], xn[:], cts, dst)

            # W_qkv^T strips + Q^T/K^T/V projections
            vw = None
            for wt in range(3 * C // 128):  # 24 feature row-tiles
                wn = wnat.tile([128, C], F32)
                nc.sync.dma_start(wn[:], w_qkv[wt * 128:(wt + 1) * 128, :])
                if wt < 16:
                    # Q (wt<8) / K rows: strip [c_part, ct, 128 feats]
                    ws = wstrip.tile([128, CT * 128], F32R)
                    for g in range(2):
                        dst = ws[:].rearrange("p (ct f) -> p ct f", ct=CT)[
                            :, g * 4:g * 4 + 4, :]
                        _transpose_group(nc, tp_ps, ident[:], wn[:],
                                         range(g * 4, g * 4 + 4), dst)
                    if wt < 8:
                        nsch, s_base, dest, drow = 2, "q", qT_d, wt
                    else:
                        nsch, s_base, dest, drow = 4, "k", kT_d, wt - 8
                    for sch in range(nsch):
                        acc = acc_ps.tile([128, 512], F32)
                        for ct in range(CT):
                            # Q uses rows [0, MYROWS) of the (host-rolled)
                            # sequence; K uses the full sequence.
                            soff = sch * 512
                            nc.tensor.matmul(
                                acc[:],
                                ws[:, ct * 128:(ct + 1) * 128],
                                xT[:, ct * S + soff: ct * S + soff + 512],
                                start=(ct == 0), stop=(ct == CT - 1))
                        stg = stage.tile([128, 512], F32R)
                        nc.vector.tensor_scalar_add(stg[:], acc[:],
                                                    b_sb[:, wt:wt + 1])
                        nc.sync.dma_start(
                            dest[drow * 128:(drow + 1) * 128,
                                 sch * 512:(sch + 1) * 512], stg[:])
                else:
                    # V rows: build W_v^T chunk [c_part, ct, 512 feats]
                    vt_local = wt - 16
                    vch, k4 = vt_local // 4, vt_local % 4
                    if k4 == 0:
                        vw = vw_pool.tile([128, CT * 512], F32R)
                    for g in range(2):
                        dst = vw[:].rearrange("p (ct f) -> p ct f", ct=CT)[
                            :, g * 4:g * 4 + 4, k4 * 128:(k4 + 1) * 128]
                        _transpose_group(nc, tp_ps, ident[:], wn[:],
                                         range(g * 4, g * 4 + 4), dst)
                    if k4 == 3:
                        for st in range(ST):
                            acc = acc_ps.tile([128, 512], F32)
                            for ct in range(CT):
                                nc.tensor.matmul(
                                    acc[:],
                                    xT[:, ct * S + st * 128: ct * S + (st + 1) * 128],
                                    vw[:, ct * 512:(ct + 1) * 512],
                                    start=(ct == 0), stop=(ct == CT - 1))
                            stg = stage.tile([128, 512], F32R)
                            nc.vector.tensor_copy(stg[:], acc[:])
                            nc.sync.dma_start(
                                v_d[st * 128:(st + 1) * 128,
                                    vch * 512:(vch + 1) * 512], stg[:])

        # ---------------- Phase B: attention ----------------
        OT = persist.tile([128, CT * MYROWS], F32R)  # OT[p, ct*1024 + i]
        with ExitStack() as bctx:
            kp = bctx.enter_context(tc.tile_pool(name="kp", bufs=2))
            vp = bctx.enter_context(tc.tile_pool(name="vp", bufs=2))
            qp = bctx.enter_context(tc.tile_pool(name="qp", bufs=2))
            pp = bctx.enter_context(tc.tile_pool(name="pp", bufs=3))
            smalls = bctx.enter_context(tc.tile_pool(name="smalls", bufs=2))
            sc_ps = bctx.enter_context(
                tc.tile_pool(name="sc_ps", bufs=2, space="PSUM"))
            pv_ps = bctx.enter_context(
                tc.tile_pool(name="pv_ps", bufs=2, space="PSUM"))

            for h in range(H):
                kt = kp.tile([64, S], F32R)
                nc.sync.dma_start(kt[:], kT_d[h * 64:(h + 1) * 64, :])
                # V tile padded to 128 stationary columns per j-tile:
                # [V(64) | ones(1) | zeros(63)] -- full-width weights keep the
                # fp32r fast-weight-load path (128-col FWL), and the ones
                # column accumulates the softmax denominator at out row 64.
                vt = vp.tile([128, ST * 128], F32R)
                vt3 = vt[:].rearrange("p (t f) -> p t f", f=128)
                nc.vector.tensor_scalar(
                    vt3[:, :, DH:DH + 1], ident[:, 0:ST], 0.0, 1.0,
                    mybir.AluOpType.mult, mybir.AluOpType.add)
                nc.vector.tensor_scalar(
                    vt3[:, :, DH + 1:128], bo_bc[:, 0:ST * (127 - DH)], 0.0, 0.0,
                    mybir.AluOpType.mult, mybir.AluOpType.mult)
                nc.sync.dma_start(
                    vt3[:, :, 0:DH],
                    v_d.rearrange("(t p) f -> p t f", p=128)[:, :, h * 64:(h + 1) * 64])
                for ich in range(2):
                    qt = qp.tile([64, 512], F32R)
                    nc.sync.dma_start(
                        qt[:], qT_d[h * 64:(h + 1) * 64, ich * 512:(ich + 1) * 512])
                    pv = pv_ps.tile([128, 512], F32)
                    # j-tiles in blocks of 3 to keep same-shape matmul runs
                    # long (shape switches stall the PE weight path)
                    jb = 0
                    for blk in (3, 3, 3, 3, 2, 2):
                        js = list(range(jb, jb + blk))
                        jb += blk
                        sc = sc_ps.tile([128, 3 * 512], F32)
                        for idx, j in enumerate(js):
                            nc.tensor.matmul(
                                sc[:, idx * 512:(idx + 1) * 512],
                                kt[:, j * 128:(j + 1) * 128], qt[:],
                                start=True, stop=True)
                        pg = pp.tile([128, 3 * 512], F32R)
                        nc.scalar.activation(pg[:, 0:blk * 512],
                                             sc[:, 0:blk * 512],
                                             AF.Exp, scale=SCALE)
                        for idx, j in enumerate(js):
                            nc.tensor.matmul(
                                pv[:], vt[:, j * 128:(j + 1) * 128],
                                pg[:, idx * 512:(idx + 1) * 512],
                                start=(j == 0), stop=(j == 15))
                    rec = smalls.tile([1, 512], F32)
                    nc.vector.reciprocal(rec[:], pv[64:65, :])
                    rb = smalls.tile([64, 512], F32)
                    nc.gpsimd.partition_broadcast(rb[:], rec[0:1, :])
                    oslice = OT[(h % 2) * 64:(h % 2) * 64 + 64,
                                (h // 2) * MYROWS + ich * 512:
                                (h // 2) * MYROWS + (ich + 1) * 512]
                    nc.vector.tensor_mul(oslice, pv[0:64, :], rb[:])
                    nc.vector.tensor_scalar_add(
                        oslice, oslice,
                        b_sb[(h % 2) * 64:(h % 2) * 64 + 64,
                             16 + h // 2:17 + h // 2])

        # ---------------- Phase C: out projection ----------------
        with ExitStack() as cctx:
            won = cctx.enter_context(tc.tile_pool(name="won", bufs=2))
            woT_pool = cctx.enter_context(tc.tile_pool(name="woT", bufs=1))
            yt_pool = cctx.enter_context(tc.tile_pool(name="yt", bufs=3))
            tp2_ps = cctx.enter_context(
                tc.tile_pool(name="tp2_ps", bufs=2, space="PSUM"))
            y_ps = cctx.enter_context(
                tc.tile_pool(name="y_ps", bufs=2, space="PSUM"))

            for et in range(2):
                woT = woT_pool.tile([128, CT * 512], F32R)  # [c_p, ct, 512 e]
                for rt in range(4):
                    wn = won.tile([128, C], F32)
                    nc.sync.dma_start(
                        wn[:], w_out[et * 512 + rt * 128: et * 512 + (rt + 1) * 128, :])
                    for g in range(2):
                        dst = woT[:].rearrange("p (ct e) -> p ct e", ct=CT)[
                            :, g * 4:g * 4 + 4, rt * 128:(rt + 1) * 128]
                        _transpose_group(nc, tp2_ps, ident[:], wn[:],
                                         range(g * 4, g * 4 + 4), dst)
                for it in range(8):
                    y = y_ps.tile([128, 512], F32)
                    for ct in range(CT):
                        nc.tensor.matmul(
                            y[:],
                            OT[:, ct * MYROWS + it * 128: ct * MYROWS + (it + 1) * 128],
                            woT[:, ct * 512:(ct + 1) * 512],
                            start=(ct == 0), stop=(ct == CT - 1))
                    yt = yt_pool.tile([128, 512], F32)
                    nc.vector.tensor_add(yt[:], y[:], bo_bc[:, et * 512:(et + 1) * 512])
                    nc.sync.dma_start(
                        out[it * 128:(it + 1) * 128, et * 512:(et + 1) * 512], yt[:])

    nc.compile()
    return nc


_cache = {}


def _get_nc():
    if "nc" not in _cache:
        _cache["nc"] = build()
    return _cache["nc"]


def kernel(x_q, W_qkv, b_qkv, W_out, b_out):
    """Core c of 8 handles batch c//2, query rows [(c%2)*1024, +1024).

    The per-core x slice is ROLLED by the core's query-row offset so every
    core's own query rows sit at rows [0, MYROWS) of its slice. Attention is
    permutation-invariant over keys, so the rolled K/V ordering does not
    change the output.
    """
    x_q = np.ascontiguousarray(x_q, dtype=np.float32)
    W_qkv = np.ascontiguousarray(W_qkv, dtype=np.float32)
    b_qkv = np.ascontiguousarray(b_qkv, dtype=np.float32)
    W_out = np.ascontiguousarray(W_out, dtype=np.float32)
    b_out = np.ascontiguousarray(b_out, dtype=np.float32)

    nc = _get_nc()
    in_maps = []
    for c in range(NCORES):
        b, half = c // 2, c % 2
        xb = x_q[b]
        if half:
            xb = np.ascontiguousarray(np.roll(xb, -MYROWS, axis=0))
        in_maps.append({
            "x": xb,
            "w_qkv": W_qkv,
            "b_qkv": b_qkv,
            "w_out": W_out,
            "b_out": b_out,
        })
    res = run_bass_kernel_spmd(nc, in_maps, list(range(NCORES)))
    out = np.empty((B, S, C), dtype=np.float32)
    for c in range(NCORES):
        b, half = c // 2, c % 2
        out[b, half * MYROWS:(half + 1) * MYROWS] = res.results[c]["out"]
    return out


if __name__ == "__main__":
    # smoke test with random inputs
    rng = np.random.default_rng(0)
    x_q = rng.standard_normal((B, S, C), dtype=np.float32)
    s = 1.0 / np.sqrt(C)
    W_qkv = rng.uniform(-s, s, (3 * C, C)).astype(np.float32)
    b_qkv = rng.uniform(-s, s, 3 * C).astype(np.float32)
    W_out = rng.uniform(-s, s, (C, C)).astype(np.float32)
    b_out = rng.uniform(-s, s, C).astype(np.float32)
    got = kernel(x_q=x_q, W_qkv=W_qkv, b_qkv=b_qkv, W_out=W_out, b_out=b_out)
    print("smoke ok", got.shape, float(np.abs(got).max()))


# revision 13
# speedup vs baseline: 1.0538x; 1.0538x over previous
"""Multi-head attention (B=4, S=2048, C=1024, H=16) on 8 TRN2 NeuronCores.

Sharding: data-parallel over batch (4) x query-row split (2). Core c handles
batch c//2, query rows [(c%2)*1024, (c%2)*1024+1024). Each core computes the
QKV projection for its batch (K/V over the full sequence, Q over its own rows)
with float32r matmuls, spills Q^T/K^T/V to DRAM scratch, then runs per-head
attention with transposed scores ([key, query] layout) so the softmaxed
probabilities feed the P.V matmul directly as the moving operand. A ones
column appended to V accumulates the softmax denominator in the same PSUM
tile. The out-projection uses O^T as the stationary operand so the result is
produced in natural [row, channel] layout. No collectives.
"""

from contextlib import ExitStack

import numpy as np

import concourse.bass as bass
import concourse.mybir as mybir
import concourse.tile as tile
from concourse import bacc
from concourse.bass_utils import run_bass_kernel_spmd
from concourse.masks import make_identity

F32 = mybir.dt.float32
F32R = mybir.dt.float32r
AF = mybir.ActivationFunctionType

B, S, C, H, DH = 4, 2048, 1024, 16, 64
NCORES = 8
SCALE = DH ** -0.5  # 0.125
CT = C // 128  # 8 channel tiles
ST = S // 128  # 16 seq tiles
MYROWS = S // 2  # 1024 query rows per core


def _transpose_group(nc, tp_pool, ident, src, cts, dst_ap):
    """PE-transpose 4 [128,128] blocks of src (channel tiles cts) and copy the
    [128, 512] group to dst_ap (a strided AP covering the 4 destinations)."""
    tp = tp_pool.tile([128, 512], F32)
    for k, ct in enumerate(cts):
        nc.tensor.transpose(tp[:, k * 128:(k + 1) * 128],
                            src[:, ct * 128:(ct + 1) * 128], ident)
    nc.vector.tensor_copy(dst_ap, tp[:])


def build():
    nc = bacc.Bacc("TRN2", target_bir_lowering=False, debug=False,
                   num_devices=NCORES)

    x = nc.dram_tensor("x", [S, C], F32, kind="ExternalInput").ap()
    w_qkv = nc.dram_tensor("w_qkv", [3 * C, C], F32, kind="ExternalInput").ap()
    b_qkv = nc.dram_tensor("b_qkv", [3 * C], F32, kind="ExternalInput").ap()
    w_out = nc.dram_tensor("w_out", [C, C], F32, kind="ExternalInput").ap()
    b_out = nc.dram_tensor("b_out", [C], F32, kind="ExternalInput").ap()
    out = nc.dram_tensor("out", [MYROWS, C], F32, kind="ExternalOutput").ap()

    # DRAM scratch for the projected tensors (feature-major Q^T/K^T, natural V)
    qT_d = nc.dram_tensor("qT_d", [C, MYROWS], F32R).ap()
    kT_d = nc.dram_tensor("kT_d", [C, S], F32R).ap()
    v_d = nc.dram_tensor("v_d", [S, C], F32R).ap()

    with tile.TileContext(nc) as tc, ExitStack() as ctx:
        const = ctx.enter_context(tc.tile_pool(name="const", bufs=1))
        ident = const.tile([128, 128], F32)
        make_identity(nc, ident[:])

        b_sb = const.tile([128, 3 * C // 128], F32)  # b_sb[p, wt] = b_qkv[wt*128+p]
        nc.sync.dma_start(b_sb[:], b_qkv.rearrange("(wt p) -> p wt", p=128))
        bo_sb = const.tile([1, C], F32)
        nc.sync.dma_start(bo_sb[:], b_out[None, :])
        bo_bc = const.tile([128, C], F32)
        nc.gpsimd.partition_broadcast(bo_bc[:], bo_sb[0:1, :])

        persist = ctx.enter_context(tc.tile_pool(name="persist", bufs=1))
        xT = persist.tile([128, CT * S], F32R)  # xT[p, ct*S + s] = x[s, ct*128+p]

        # ---------------- Phase A: transposes + QKV projection ----------------
        with ExitStack() as actx:
            xnat = actx.enter_context(tc.tile_pool(name="xnat", bufs=3))
            wnat = actx.enter_context(tc.tile_pool(name="wnat", bufs=3))
            wstrip = actx.enter_context(tc.tile_pool(name="wstrip", bufs=3))
            vw_pool = actx.enter_context(tc.tile_pool(name="vw", bufs=1))
            stage = actx.enter_context(tc.tile_pool(name="stage", bufs=4))
            tp_ps = actx.enter_context(
                tc.tile_pool(name="tp_ps", bufs=3, space="PSUM"))
            acc_ps = actx.enter_context(
                tc.tile_pool(name="acc_ps", bufs=2, space="PSUM"))

            # x^T
            for st in range(ST):
                xn = xnat.tile([128, C], F32)
                nc.sync.dma_start(xn[:], x[st * 128:(st + 1) * 128, :])
                for g in range(2):
                    cts = range(g * 4, g * 4 + 4)
                    # dst: xT[:, ct*S + st*128 : +128] for ct in cts
                    dst = xT[:].rearrange("p (ct s) -> p ct s", ct=CT)[
                        :, g * 4:g * 4 + 4, st * 128:(st + 1) * 128]
                    _transpose_group(nc, tp_ps, ident[:], xn[:], cts, dst)

            # W_qkv^T strips + Q^T/K^T/V projections
            vw = None
            for wt in range(3 * C // 128):  # 24 feature row-tiles
                wn = wnat.tile([128, C], F32)
                nc.sync.dma_start(wn[:], w_qkv[wt * 128:(wt + 1) * 128, :])
                if wt < 16:
                    # Q (wt<8) / K rows: strip [c_part, ct, 128 feats]
                    ws = wstrip.tile([128, CT * 128], F32R)
                    for g in range(2):
                        dst = ws[:].rearrange("p (ct f) -> p ct f", ct=CT)[
                            :, g * 4:g * 4 + 4, :]
                        _transpose_group(nc, tp_ps, ident[:], wn[:],
                                         range(g * 4, g * 4 + 4), dst)
                    if wt < 8:
                        nsch, s_base, dest, drow = 2, "q", qT_d, wt
                    else:
                        nsch, s_base, dest, drow = 4, "k", kT_d, wt - 8
                    for sch in range(nsch):
                        acc = acc_ps.tile([128, 512], F32)
                        for ct in range(CT):
                            # Q uses rows [0, MYROWS) of the (host-rolled)
                            # sequence; K uses the full sequence.
                            soff = sch * 512
                            nc.tensor.matmul(
                                acc[:],
                                ws[:, ct * 128:(ct + 1) * 128],
                                xT[:, ct * S + soff: ct * S + soff + 512],
                                start=(ct == 0), stop=(ct == CT - 1))
                        stg = stage.tile([128, 512], F32R)
                        nc.vector.tensor_scalar_add(stg[:], acc[:],
                                                    b_sb[:, wt:wt + 1])
                        nc.sync.dma_start(
                            dest[drow * 128:(drow + 1) * 128,
                                 sch * 512:(sch + 1) * 512], stg[:])
                else:
                    # V rows: build W_v^T chunk [c_part, ct, 512 feats]
                    vt_local = wt - 16
                    vch, k4 = vt_local // 4, vt_local % 4
                    if k4 == 0:
                        vw = vw_pool.tile([128, CT * 512], F32R)
                    for g in range(2):
                        dst = vw[:].rearrange("p (ct f) -> p ct f", ct=CT)[
                            :, g * 4:g * 4 + 4, k4 * 128:(k4 + 1) * 128]
                        _transpose_group(nc, tp_ps, ident[:], wn[:],
                                         range(g * 4, g * 4 + 4), dst)
                    if k4 == 3:
                        for st in range(ST):
                            acc = acc_ps.tile([128, 512], F32)
                            for ct in range(CT):
                                nc.tensor.matmul(
                                    acc[:],
                                    xT[:, ct * S + st * 128: ct * S + (st + 1) * 128],
                                    vw[:, ct * 512:(ct + 1) * 512],
                                    start=(ct == 0), stop=(ct == CT - 1))
                            stg = stage.tile([128, 512], F32R)
                            nc.vector.tensor_copy(stg[:], acc[:])
                            nc.sync.dma_start(
                                v_d[st * 128:(st + 1) * 128,
                                    vch * 512:(vch + 1) * 512], stg[:])

        # ---------------- Phase B: attention ----------------
        OT = persist.tile([128, CT * MYROWS], F32R)  # OT[p, ct*1024 + i]
        with ExitStack() as bctx:
            kp = bctx.enter_context(tc.tile_pool(name="kp", bufs=2))
            vp = bctx.enter_context(tc.tile_pool(name="vp", bufs=1))
            qp = bctx.enter_context(tc.tile_pool(name="qp", bufs=2))
            pp = bctx.enter_context(tc.tile_pool(name="pp", bufs=2))
            smalls = bctx.enter_context(tc.tile_pool(name="smalls", bufs=2))
            sc_ps = bctx.enter_context(
                tc.tile_pool(name="sc_ps", bufs=1, space="PSUM"))
            pv_ps = bctx.enter_context(
                tc.tile_pool(name="pv_ps", bufs=1, space="PSUM"))

            for hp in range(H // 2):  # head pairs: A = rows 0-63, B = 64-127
                kt = kp.tile([128, S], F32R)
                nc.sync.dma_start(kt[:], kT_d[hp * 128:(hp + 1) * 128, :])
                vts = []
                for half in range(2):
                    # V tile padded to 128 stationary columns per j-tile:
                    # [V(64) | ones(1) | zeros(63)] -- full-width weights keep
                    # the fp32r fast-weight-load path (128-col FWL); the ones
                    # column accumulates the softmax denominator at out row 64.
                    vt = vp.tile([128, ST * 128], F32R, tag=f"vt{half}")
                    vt3 = vt[:].rearrange("p (t f) -> p t f", f=128)
                    nc.vector.tensor_scalar(
                        vt3[:, :, DH:DH + 1], ident[:, 0:ST], 0.0, 1.0,
                        mybir.AluOpType.mult, mybir.AluOpType.add)
                    nc.vector.tensor_scalar(
                        vt3[:, :, DH + 1:128], bo_bc[:, 0:ST * (127 - DH)],
                        0.0, 0.0, mybir.AluOpType.mult, mybir.AluOpType.mult)
                    h = hp * 2 + half
                    nc.sync.dma_start(
                        vt3[:, :, 0:DH],
                        v_d.rearrange("(t p) f -> p t f", p=128)[
                            :, :, h * 64:(h + 1) * 64])
                    vts.append(vt)
                for ich in range(2):
                    qt = qp.tile([128, 512], F32R)
                    nc.sync.dma_start(
                        qt[:], qT_d[hp * 128:(hp + 1) * 128,
                                    ich * 512:(ich + 1) * 512])
                    pvs = [pv_ps.tile([128, 512], F32, tag=f"pv{half}",
                                      name=f"pv{half}")
                           for half in range(2)]
                    jb = 0
                    for blk in (3, 3, 3, 3, 2, 2):
                        js = list(range(jb, jb + blk))
                        jb += blk
                        scs = [sc_ps.tile([128, 3 * 512], F32, tag=f"sc{half}",
                                          name=f"sc{half}")
                               for half in range(2)]
                        for idx, j in enumerate(js):
                            # row-packed pair: head A on PE rows 0-63,
                            # head B on rows 64-127, concurrent
                            for half in range(2):
                                p0 = half * 64
                                nc.tensor.matmul(
                                    scs[half][:, idx * 512:(idx + 1) * 512],
                                    kt[p0:p0 + 64, j * 128:(j + 1) * 128],
                                    qt[p0:p0 + 64, :],
                                    start=True, stop=True)
                        pgs = []
                        for half in range(2):
                            pg = pp.tile([128, 3 * 512], F32R, tag=f"pg{half}")
                            nc.scalar.activation(pg[:, 0:blk * 512],
                                                 scs[half][:, 0:blk * 512],
                                                 AF.Exp, scale=SCALE)
                            pgs.append(pg)
                        for half in range(2):
                            for idx, j in enumerate(js):
                                nc.tensor.matmul(
                                    pvs[half][:],
                                    vts[half][:, j * 128:(j + 1) * 128],
                                    pgs[half][:, idx * 512:(idx + 1) * 512],
                                    start=(j == 0), stop=(j == 15))
                    for half in range(2):
                        pv = pvs[half]
                        rec = smalls.tile([1, 512], F32)
                        nc.vector.reciprocal(rec[:], pv[64:65, :])
                        rb = smalls.tile([64, 512], F32)
                        nc.gpsimd.partition_broadcast(rb[:], rec[0:1, :])
                        oslice = OT[half * 64:half * 64 + 64,
                                    hp * MYROWS + ich * 512:
                                    hp * MYROWS + (ich + 1) * 512]
                        nc.vector.tensor_mul(oslice, pv[0:64, :], rb[:])
                        nc.vector.tensor_scalar_add(
                            oslice, oslice,
                            b_sb[half * 64:half * 64 + 64, 16 + hp:17 + hp])

        # ---------------- Phase C: out projection ----------------
        with ExitStack() as cctx:
            won = cctx.enter_context(tc.tile_pool(name="won", bufs=2))
            woT_pool = cctx.enter_context(tc.tile_pool(name="woT", bufs=1))
            yt_pool = cctx.enter_context(tc.tile_pool(name="yt", bufs=3))
            tp2_ps = cctx.enter_context(
                tc.tile_pool(name="tp2_ps", bufs=2, space="PSUM"))
            y_ps = cctx.enter_context(
                tc.tile_pool(name="y_ps", bufs=2, space="PSUM"))

            for et in range(2):
                woT = woT_pool.tile([128, CT * 512], F32R)  # [c_p, ct, 512 e]
                for rt in range(4):
                    wn = won.tile([128, C], F32)
                    nc.sync.dma_start(
                        wn[:], w_out[et * 512 + rt * 128: et * 512 + (rt + 1) * 128, :])
                    for g in range(2):
                        dst = woT[:].rearrange("p (ct e) -> p ct e", ct=CT)[
                            :, g * 4:g * 4 + 4, rt * 128:(rt + 1) * 128]
                        _transpose_group(nc, tp2_ps, ident[:], wn[:],
                                         range(g * 4, g * 4 + 4), dst)
                for it in range(8):
                    y = y_ps.tile([128, 512], F32)
                    for ct in range(CT):
                        nc.tensor.matmul(
                            y[:],
                            OT[:, ct * MYROWS + it * 128: ct * MYROWS + (it + 1) * 128],
                            woT[:, ct * 512:(ct + 1) * 512],
                            start=(ct == 0), stop=(ct == CT - 1))
                    yt = yt_pool.tile([128, 512], F32)
                    nc.vector.tensor_add(yt[:], y[:], bo_bc[:, et * 512:(et + 1) * 512])
                    nc.sync.dma_start(
                        out[it * 128:(it + 1) * 128, et * 512:(et + 1) * 512], yt[:])

    nc.compile()
    return nc


_cache = {}


def _get_nc():
    if "nc" not in _cache:
        _cache["nc"] = build()
    return _cache["nc"]


def kernel(x_q, W_qkv, b_qkv, W_out, b_out):
    """Core c of 8 handles batch c//2, query rows [(c%2)*1024, +1024).

    The per-core x slice is ROLLED by the core's query-row offset so every
    core's own query rows sit at rows [0, MYROWS) of its slice. Attention is
    permutation-invariant over keys, so the rolled K/V ordering does not
    change the output.
    """
    x_q = np.ascontiguousarray(x_q, dtype=np.float32)
    W_qkv = np.ascontiguousarray(W_qkv, dtype=np.float32)
    b_qkv = np.ascontiguousarray(b_qkv, dtype=np.float32)
    W_out = np.ascontiguousarray(W_out, dtype=np.float32)
    b_out = np.ascontiguousarray(b_out, dtype=np.float32)

    nc = _get_nc()
    in_maps = []
    for c in range(NCORES):
        b, half = c // 2, c % 2
        xb = x_q[b]
        if half:
            xb = np.ascontiguousarray(np.roll(xb, -MYROWS, axis=0))
        in_maps.append({
            "x": xb,
            "w_qkv": W_qkv,
            "b_qkv": b_qkv,
            "w_out": W_out,
            "b_out": b_out,
        })
    res = run_bass_kernel_spmd(nc, in_maps, list(range(NCORES)))
    out = np.empty((B, S, C), dtype=np.float32)
    for c in range(NCORES):
        b, half = c // 2, c % 2
        out[b, half * MYROWS:(half + 1) * MYROWS] = res.results[c]["out"]
    return out


if __name__ == "__main__":
    # smoke test with random inputs
    rng = np.random.default_rng(0)
    x_q = rng.standard_normal((B, S, C), dtype=np.float32)
    s = 1.0 / np.sqrt(C)
    W_qkv = rng.uniform(-s, s, (3 * C, C)).astype(np.float32)
    b_qkv = rng.uniform(-s, s, 3 * C).astype(np.float32)
    W_out = rng.uniform(-s, s, (C, C)).astype(np.float32)
    b_out = rng.uniform(-s, s, C).astype(np.float32)
    got = kernel(x_q=x_q, W_qkv=W_qkv, b_qkv=b_qkv, W_out=W_out, b_out=b_out)
    print("smoke ok", got.shape, float(np.abs(got).max()))
